# revision 15
# baseline (speedup 1.0000x reference)
"""AdaAttention Trainium2 kernel — data-parallel over batch across 8 NeuronCores.

Full shapes: h [1024,512], sentinel [1024,512], att_feats [1024,96,2048] -> out [1024,512].
Per core: b=128 batch rows. Token axis x = s*128 + b (s-major), N_tok = 12288.

v8: fp8 DoubleRow matmuls + online exp-weighted cHat, 6-stage decoupled pipeline.
Per iteration i (each cross-engine edge >= ~1 iteration of slack):
  tr(i+1):   sync: 4 u16 xbar transposes nat->attf (HWDGE kept nearly exclusive)
  mm(i):     PE: MM1 32 DR-MMs; ACT: relu -> attT fp8
  b_mm(i-1): PE: MM2 8 DR-MMs + 4 ident-fold MMs (adds h_eT' via identity selection);
             ACT: tanh((...)/64 + b_c+b_h) -> hat fp8
  logit(i-2): PE: 2 DR logit MMs; ACT: exp(x/16) -> e_row bf16; PE: col transposes
             + ones-bcast -> e_rep psum; DVE: e_sb copy
  chat(i-3): DVE: prod = attT*e_rep; pair-tree adds; cacc += .
  in(i+2):   gpsimd: one 3MB f32 DMA; scalar: 1MB f32 DMA; casts f32->fp8: 3 DVE + 1 ACT
Weights prescaled (W_ae x128, W_c/W_s/W_h x64, W_al x16) to dodge fp8 subnormals;
descale folded into ACT scale. b_al dropped (cancels in exp-weight normalization).
fp8 f-mapping through the u16 xbar: f = 256*fb + 2*p + parity; host permutes weights.
tail: ssum = reduce(e_sb); rinv; transpose+bcast; attenT = cacc*rinv_rep + hT_bf;
      out = tanh(attenT @ w_o + b_o); PE transpose; store.
"""
import sys

for p in ("/opt/trn_rl_repo", "/opt/pypackages"):
    if p not in sys.path:
        sys.path.insert(0, p)

import numpy as np
import ml_dtypes
from contextlib import ExitStack

import concourse.bass as bass
import concourse.bacc as bacc
import concourse.mybir as mybir
from concourse import tile

F32 = mybir.dt.float32
BF16 = mybir.dt.bfloat16
F8 = mybir.dt.float8e4
AF = mybir.ActivationFunctionType
ALU = mybir.AluOpType
DR = mybir.MatmulPerfMode.DoubleRow

NCORES = 8
B_LOC = 128          # batch rows per core
S = 96               # attention slots
F = 2048             # att feature size
R = 512              # rnn size
A = 512              # att hidden size
NTOK = B_LOC * S     # 12288
XCHUNK = 512         # tokens per pipeline chunk (4 s-tiles)
NCHUNKS = NTOK // XCHUNK       # 24
S_PER_CHUNK = XCHUNK // B_LOC  # 4
FB = F // 256        # 8 f-pair-blocks
RT = R // 128        # 4
AT = A // 128        # 4

W_AE_SCALE = 128.0
W_C_SCALE = 64.0
W_AL_SCALE = 16.0


def build_nc():
    nc = bacc.Bacc("TRN2", target_bir_lowering=False, debug=False)

    # ---- DRAM parameters (per-core shard shapes) ----
    att_feats = nc.declare_dram_parameter("att_feats", [B_LOC, S, F], F32, isOutput=False)
    h_in = nc.declare_dram_parameter("h", [B_LOC, R], F32, isOutput=False)
    sent_in = nc.declare_dram_parameter("sentinel", [B_LOC, R], F32, isOutput=False)
    # fp8 weights, host-permuted for DoubleRow (see prep_shared)
    w_ae_d = nc.declare_dram_parameter("w_ae", [128, FB, 2, R], F8, isOutput=False)
    w_c_d = nc.declare_dram_parameter("w_c", [128, 2, 2, A], F8, isOutput=False)
    w_s_d = nc.declare_dram_parameter("w_s", [128, 2, 2, A], F8, isOutput=False)
    w_h_d = nc.declare_dram_parameter("w_h", [128, 2, 2, A], F8, isOutput=False)
    w_al_d = nc.declare_dram_parameter("w_al", [128, 2, 2, 16], F8, isOutput=False)
    w_o_d = nc.declare_dram_parameter("w_o", [128, RT, R], BF16, isOutput=False)
    b_ae_d = nc.declare_dram_parameter("b_ae", [128, RT], F32, isOutput=False)
    b_ch_d = nc.declare_dram_parameter("b_ch", [128, AT], F32, isOutput=False)
    b_sh_d = nc.declare_dram_parameter("b_sh", [128, AT], F32, isOutput=False)
    b_o_d = nc.declare_dram_parameter("b_o", [128, RT], F32, isOutput=False)
    ident_d = nc.declare_dram_parameter("ident", [128, 128], BF16, isOutput=False)
    ident_f32_d = nc.declare_dram_parameter("ident_f32", [128, 128], F32, isOutput=False)
    ones_d = nc.declare_dram_parameter("ones_row", [1, 128], BF16, isOutput=False)
    out_d = nc.declare_dram_parameter("out", [B_LOC, R], F32, isOutput=True)

    with tile.TileContext(nc) as tc, ExitStack() as ctx:
        # ---- pools ----
        cp = ctx.enter_context(tc.tile_pool(name="consts", bufs=1))
        nat_p = ctx.enter_context(tc.tile_pool(name="nat", bufs=4))
        attf_p = ctx.enter_context(tc.tile_pool(name="attf", bufs=3))
        attT_p = ctx.enter_context(tc.tile_pool(name="attT", bufs=6))
        stg_p = ctx.enter_context(tc.tile_pool(name="stg", bufs=8))
        hat_p = ctx.enter_context(tc.tile_pool(name="hat", bufs=3))
        prod_p = ctx.enter_context(tc.tile_pool(name="prod", bufs=1))
        small_p = ctx.enter_context(tc.tile_pool(name="small", bufs=2))
        erow_p = ctx.enter_context(tc.tile_pool(name="erow", bufs=3))
        soft_p = ctx.enter_context(tc.tile_pool(name="soft", bufs=3))
        ps_mm1 = ctx.enter_context(tc.tile_pool(name="ps_mm1", bufs=2, space="PSUM"))
        ps_mm2 = ctx.enter_context(tc.tile_pool(name="ps_mm2", bufs=2, space="PSUM"))
        ps_rep = ctx.enter_context(tc.tile_pool(name="ps_rep", bufs=2, space="PSUM"))
        ps_small = ctx.enter_context(tc.tile_pool(name="ps_small", bufs=2, space="PSUM"))

        nat_tiles = {}
        attf_chunks = {}
        attT_chunks = {}
        hat_chunks = {}
        erep_psum = {}

        def const_tile(name, shape, dtype, src):
            t = cp.tile(shape, dtype, tag=name, name=name)
            nc.scalar.dma_start(out=t[:], in_=src[:])
            return t

        # w_ae + small consts first so chunk-0 matmuls can start ASAP
        w_ae = const_tile("w_ae", [128, FB, 2, R], F8, w_ae_d)
        b_ae = const_tile("b_ae", [128, RT], F32, b_ae_d)
        ident = const_tile("ident", [128, 128], BF16, ident_d)
        ident_f32 = const_tile("ident_f32", [128, 128], F32, ident_f32_d)
        ones_row = const_tile("ones_row", [1, 128], BF16, ones_d)

        # h / sentinel early (tiny SWDGE cast-DMAs; gpsimd queue is otherwise idle)
        h_bf = cp.tile([B_LOC, R], BF16, tag="h_bf", name="h_bf")
        nc.gpsimd.dma_start(out=h_bf[:], in_=h_in[:])
        sent_bf = cp.tile([B_LOC, R], BF16, tag="sent_bf", name="sent_bf")
        nc.gpsimd.dma_start(out=sent_bf[:], in_=sent_in[:])

        def stage_in(c, preloop=False):
            # per-slice 1MB plain f32 loads: slices 0-2 SWDGE (gpsimd), slice 3
            # HWDGE (scalar) -- except pre-loop chunks, all-SWDGE so the xbar
            # stream is not serialized behind them (HWDGE transpose lock).
            # f32->fp8 casts: slices 0,1,3 on DVE; slice 2 on ACT.
            s0 = c * S_PER_CHUNK
            nat = nat_p.tile([B_LOC, S_PER_CHUNK, F], F8, tag="nat", name=f"nat_{c}")
            stgs = []
            for i in range(S_PER_CHUNK):
                stg = stg_p.tile([B_LOC, F], F32, tag="stg", name=f"stg_{c}_{i}")
                if i == 3 and not preloop:
                    nc.scalar.dma_start(out=stg[:], in_=att_feats[:, s0 + i, :])
                else:
                    nc.gpsimd.dma_start(out=stg[:], in_=att_feats[:, s0 + i, :])
                stgs.append(stg)
            nc.vector.tensor_copy(nat[:, 0, :], stgs[0][:])
            nc.vector.tensor_copy(nat[:, 1, :], stgs[1][:])
            nc.scalar.activation(nat[:, 2, :], stgs[2][:], AF.Copy)
            nc.vector.tensor_copy(nat[:, 3, :], stgs[3][:])
            nat_tiles[c] = nat

        stage_in(0, preloop=True)
        stage_in(1, preloop=True)

        # ---- remaining constants / weights ----
        w_c = const_tile("w_c", [128, 2, 2, A], F8, w_c_d)
        w_s = const_tile("w_s", [128, 2, 2, A], F8, w_s_d)
        w_h = const_tile("w_h", [128, 2, 2, A], F8, w_h_d)
        w_o = const_tile("w_o", [128, RT, R], BF16, w_o_d)
        wal = const_tile("wal", [128, 2, 2, 16], F8, w_al_d)
        b_ch = const_tile("b_ch", [128, AT], F32, b_ch_d)
        b_sh = const_tile("b_sh", [128, AT], F32, b_sh_d)
        b_o = const_tile("b_o", [128, RT], F32, b_o_d)

        # exp'd-logit table [b, 1+S] f32 and the cHat accumulator [r_p, rb, b] f32
        e_sb = cp.tile([B_LOC, 1 + S], F32, tag="e_sb", name="e_sb")
        cacc = cp.tile([128, RT, B_LOC], F32, tag="cacc", name="cacc")

        stage_in(2, preloop=True)
        stage_in(3, preloop=True)

        def stage_tr(c):
            # u16 xbar transpose: attf[q, fb, i, j](u16) = nat_u16[b=j, i, fb*128+q]
            nat = nat_tiles.pop(c)
            attf = attf_p.tile([128, FB, S_PER_CHUNK, 128], BF16, tag="attf", name=f"attf_{c}")
            natv = nat[:].bitcast(BF16)  # [128, 4, 1024]
            for i in range(S_PER_CHUNK):
                nc.sync.dma_start(out=attf[:, :, i, :], in_=natv[:, i, :], transpose=True)
            attf_chunks[c] = attf

        stage_tr(0)
        stage_tr(1)

        prep_out = {}

        def prep():
            # transposed copies: hT_bf (final add), hT4 fp8 (replicated over i
            # for MM folds), sentT fp8 (cHat init + sentinel embed)
            hT_bf = cp.tile([128, RT, B_LOC], BF16, tag="hT_bf", name="hT_bf")
            hT4 = cp.tile([128, RT, S_PER_CHUNK, B_LOC], F8, tag="hT4", name="hT4")
            sentT = cp.tile([128, RT, B_LOC], F8, tag="sentT", name="sentT")
            for rb in range(RT):
                pt = ps_small.tile([128, 128], BF16, tag="pssm", name=f"pt_h_{rb}")
                nc.tensor.transpose(pt[:], h_bf[:, rb * 128:(rb + 1) * 128], ident[:])
                nc.vector.tensor_copy(hT_bf[:, rb, :], pt[:])
                nc.vector.tensor_copy(
                    hT4[:, rb, :, :],
                    pt[:].unsqueeze(1).broadcast_to([128, S_PER_CHUNK, B_LOC]))
                pt2 = ps_small.tile([128, 128], BF16, tag="pssm", name=f"pt_s_{rb}")
                nc.tensor.transpose(pt2[:], sent_bf[:, rb * 128:(rb + 1) * 128], ident[:])
                nc.vector.tensor_copy(sentT[:, rb, :], pt2[:])

            # h_eT' = 64*(h @ W_h.T)  (no bias; b_h folded into ACT biases)
            # then transpose -> h_eTT [b_p, ab, a] bf16 for the MM2 ident-fold
            h_eT = cp.tile([128, AT, B_LOC], BF16, tag="h_eT", name="h_eT")
            h_eTT = cp.tile([128, AT, 128], BF16, tag="h_eTT", name="h_eTT")
            for ab in range(AT):
                psh = ps_small.tile([128, B_LOC], F32, tag="pssm", name=f"psh_{ab}")
                for t in range(2):
                    nc.tensor.matmul(psh[:], w_h[:, t, :, ab * 128:(ab + 1) * 128],
                                     hT4[:, 2 * t:2 * t + 2, 0, :],
                                     start=(t == 0), stop=(t == 1), perf_mode=DR)
                nc.vector.tensor_copy(h_eT[:, ab, :], psh[:])
                ptt = ps_small.tile([128, 128], BF16, tag="pssm", name=f"ptt_{ab}")
                nc.tensor.transpose(ptt[:], h_eT[:, ab, :], ident[:])
                nc.vector.tensor_copy(h_eTT[:, ab, :], ptt[:])

            # hA_sentT = tanh((sent_e' + h_e')/64 + b_s + b_h)  [a_p, ab, b] fp8
            hA_sentT = cp.tile([128, AT, B_LOC], F8, tag="hA_sentT", name="hA_sentT")
            for ab in range(AT):
                pss = ps_small.tile([128, B_LOC], F32, tag="pssm", name=f"pss_{ab}")
                for t in range(2):
                    nc.tensor.matmul(pss[:], w_s[:, t, :, ab * 128:(ab + 1) * 128],
                                     sentT[:, 2 * t:2 * t + 2, :],
                                     start=(t == 0), stop=False, perf_mode=DR)
                for t in range(2):
                    nc.tensor.matmul(pss[:], w_h[:, t, :, ab * 128:(ab + 1) * 128],
                                     hT4[:, 2 * t:2 * t + 2, 0, :],
                                     start=False, stop=(t == 1), perf_mode=DR)
                nc.scalar.activation(hA_sentT[:, ab, :], pss[:], AF.Tanh,
                                     bias=b_sh[:, ab:ab + 1], scale=1.0 / W_C_SCALE)

            # sentinel exp'd logit -> e_sb[:, 0], and cacc init = e0 * sentT
            ps_lr0 = ps_small.tile([1, B_LOC], F32, tag="pssm", name="ps_lr0")
            for t in range(2):
                nc.tensor.matmul(ps_lr0[:], wal[:, t, :, 0:1],
                                 hA_sentT[:, 2 * t:2 * t + 2, :],
                                 start=(t == 0), stop=(t == 1), perf_mode=DR)
            e0_row = small_p.tile([1, B_LOC], BF16, tag="lrow", name="e0_row")
            nc.scalar.activation(e0_row[:], ps_lr0[:], AF.Exp, scale=1.0 / W_AL_SCALE)
            ps_ec0 = ps_small.tile([128, 2], BF16, tag="pssm", name="ps_ec0")
            nc.tensor.transpose(ps_ec0[:, 0:1], e0_row[:], ident[0:1, 0:1])
            nc.vector.tensor_copy(e_sb[:, 0:1], ps_ec0[:, 0:1])
            ps_e0rep = ps_rep.tile([128, B_LOC], F32, tag="rep", name="ps_e0rep")
            nc.tensor.matmul(ps_e0rep[:], ones_row[:], e0_row[:], start=True, stop=True)
            nc.vector.tensor_tensor(
                out=cacc[:], in0=sentT[:],
                in1=ps_e0rep[:].unsqueeze(1).broadcast_to([128, RT, B_LOC]),
                op=ALU.mult)

            prep_out.update(hT_bf=hT_bf, hT4=hT4, h_eTT=h_eTT)

        def stage_mm(c):
            attf = attf_chunks.pop(c)
            attf8 = attf[:].bitcast(F8)  # [128, FB, 4, 256]
            attT = attT_p.tile([128, RT, XCHUNK], F8, tag="attT", name=f"attT_{c}")
            for rb in range(RT):
                ps1 = ps_mm1.tile([128, XCHUNK], F32, tag="mm1", name=f"ps1_{c}_{rb}")
                for fb in range(FB):
                    rhs = attf8[:, fb, :, :].rearrange("p i (j two) -> p two (i j)", two=2)
                    nc.tensor.matmul(ps1[:], w_ae[:, fb, :, rb * 128:(rb + 1) * 128],
                                     rhs, start=(fb == 0), stop=(fb == FB - 1),
                                     perf_mode=DR)
                nc.scalar.activation(attT[:, rb, :], ps1[:], AF.Relu,
                                     bias=b_ae[:, rb:rb + 1], scale=1.0 / W_AE_SCALE)
            attT_chunks[c] = attT

        def stage_b_mm(c):
            attT = attT_chunks[c]
            h_eTT = prep_out["h_eTT"]
            # MM2 (fp8 DR) + h_eT' via identity-selection matmul (bf16) ->
            # tanh(x/64 + b_c + b_h) -> hat fp8.  No DVE in this chain.
            hat = hat_p.tile([128, AT, XCHUNK], F8, tag="hat", name=f"hat_{c}")
            for ab in range(AT):
                ps2 = ps_mm2.tile([128, XCHUNK], F32, tag="mm2", name=f"ps2_{c}_{ab}")
                for t in range(2):
                    nc.tensor.matmul(ps2[:], w_c[:, t, :, ab * 128:(ab + 1) * 128],
                                     attT[:, 2 * t:2 * t + 2, :],
                                     start=(t == 0), stop=False, perf_mode=DR)
                # += h_eT'[a, b(x)]: lhsT = h_eTT[b_p, a], rhs = identity
                # broadcast over the 4 s-slices (stride-0 dim)
                nc.tensor.matmul(
                    ps2[:], h_eTT[:, ab, :],
                    ident[:].unsqueeze(1).broadcast_to([128, S_PER_CHUNK, 128]),
                    start=False, stop=True)
                nc.scalar.activation(hat[:, ab, :], ps2[:], AF.Tanh,
                                     bias=b_ch[:, ab:ab + 1], scale=1.0 / W_C_SCALE)
            hat_chunks[c] = hat

        def stage_logit(c):
            hat = hat_chunks.pop(c)
            # logits row (fp8 DR) -> exp -> e_row bf16
            ps_l = ps_small.tile([1, XCHUNK], F32, tag="pssm", name=f"ps_l_{c}")
            for t in range(2):
                nc.tensor.matmul(ps_l[:], wal[:, t, :, 0:1],
                                 hat[:, 2 * t:2 * t + 2, :],
                                 start=(t == 0), stop=(t == 1), perf_mode=DR)
            e_row = erow_p.tile([1, XCHUNK], BF16, tag="lrow", name=f"e_row_{c}")
            nc.scalar.activation(e_row[:], ps_l[:], AF.Exp, scale=1.0 / W_AL_SCALE)

            # e columns -> e_sb table (bf16 psum, even columns for 4B alignment)
            ps_cc = ps_small.tile([128, 2 * S_PER_CHUNK], BF16, tag="pssm", name=f"ps_cc_{c}")
            for i in range(S_PER_CHUNK):
                nc.tensor.transpose(ps_cc[:, 2 * i:2 * i + 1], e_row[:, i * 128:(i + 1) * 128],
                                    ident[0:1, 0:1])
            nc.vector.tensor_copy(
                e_sb[:, 1 + c * S_PER_CHUNK: 1 + (c + 1) * S_PER_CHUNK].unsqueeze(2),
                ps_cc[:].rearrange("p (i two) -> p i two", two=2)[:, :, 0:1])

            # broadcast e across partitions: e_rep[p, i*128+b] = e_row[i*128+b]
            ps_er = ps_rep.tile([128, XCHUNK], F32, tag="rep", name=f"ps_er_{c}")
            nc.tensor.matmul(ps_er[:], ones_row[:], e_row[:], start=True, stop=True)
            erep_psum[c] = ps_er

        def stage_chat(c):
            # cacc += sum_i attT * e_rep   (DVE, pair-tree adds)
            attT = attT_chunks.pop(c)
            ps_er = erep_psum.pop(c)
            prod = prod_p.tile([128, RT, S_PER_CHUNK, B_LOC], BF16, tag="prod", name=f"prod_{c}")
            nc.vector.tensor_tensor(
                out=prod[:],
                in0=attT[:].rearrange("p rb (i j) -> p rb i j", i=S_PER_CHUNK),
                in1=ps_er[:].rearrange("p (i j) -> p i j", i=S_PER_CHUNK)
                    .unsqueeze(1).broadcast_to([128, RT, S_PER_CHUNK, B_LOC]),
                op=ALU.mult)
            t1 = prod_p.tile([128, RT, B_LOC], F32, tag="tree", name=f"t1_{c}")
            nc.vector.tensor_tensor(out=t1[:], in0=prod[:, :, 0, :], in1=prod[:, :, 1, :], op=ALU.add)
            t2 = prod_p.tile([128, RT, B_LOC], F32, tag="tree2", name=f"t2_{c}")
            nc.vector.tensor_tensor(out=t2[:], in0=prod[:, :, 2, :], in1=prod[:, :, 3, :], op=ALU.add)
            t3 = prod_p.tile([128, RT, B_LOC], F32, tag="tree3", name=f"t3_{c}")
            nc.vector.scalar_tensor_tensor(out=t3[:], in0=t1[:], scalar=0.0, in1=t2[:],
                                           op0=ALU.add, op1=ALU.add)
            nc.vector.tensor_tensor(out=cacc[:], in0=cacc[:], in1=t3[:], op=ALU.add)

        prep()

        # ---- main pipeline: tr(i+1) | mm(i) | b_mm(i-1) | logit(i-2) | chat(i-3) | in(i+2)
        for i in range(NCHUNKS + 3):
            if 2 <= i + 1 < NCHUNKS:
                stage_tr(i + 1)
            if i < NCHUNKS:
                stage_mm(i)
            if 0 <= i - 1 < NCHUNKS:
                stage_b_mm(i - 1)
            if 0 <= i - 2 < NCHUNKS:
                stage_logit(i - 2)
            if 0 <= i - 3 < NCHUNKS:
                stage_chat(i - 3)
            if 4 <= i + 2 < NCHUNKS:
                stage_in(i + 2)

        # ---- tail: normalize cHat, add h, project, store ----
        ssum = soft_p.tile([B_LOC, 1], F32, tag="soft", name="ssum")
        nc.vector.tensor_reduce(out=ssum[:], in_=e_sb[:], op=ALU.add,
                                axis=mybir.AxisListType.X)
        rinv = soft_p.tile([B_LOC, 1], F32, tag="soft", name="rinv")
        nc.vector.reciprocal(rinv[:], ssum[:])
        ps_rr = ps_small.tile([1, B_LOC], F32, tag="pssm", name="ps_rr")
        nc.tensor.transpose(ps_rr[:], rinv[:], ident_f32[:])
        rr_row = soft_p.tile([1, B_LOC], F32, tag="soft", name="rr_row")
        nc.vector.tensor_copy(rr_row[:], ps_rr[:])
        ones_f32 = soft_p.tile([1, B_LOC], F32, tag="soft2", name="ones_f32")
        nc.vector.tensor_copy(ones_f32[:], ones_row[:])
        ps_rrep = ps_rep.tile([128, B_LOC], F32, tag="rep", name="ps_rrep")
        nc.tensor.matmul(ps_rrep[:], ones_f32[:], rr_row[:], start=True, stop=True)

        hT_bf = prep_out["hT_bf"]
        attenT = cp.tile([128, RT, B_LOC], BF16, tag="attenT", name="attenT")
        nc.vector.tensor_tensor(
            out=attenT[:], in0=cacc[:],
            in1=ps_rrep[:].unsqueeze(1).broadcast_to([128, RT, B_LOC]),
            op=ALU.mult)
        nc.vector.tensor_tensor(out=attenT[:], in0=attenT[:], in1=hT_bf[:], op=ALU.add)

        for ob in range(RT):
            pso = ps_small.tile([128, B_LOC], F32, tag="pssm", name=f"pso_{ob}")
            for rb in range(RT):
                nc.tensor.matmul(pso[:], w_o[:, rb, ob * 128:(ob + 1) * 128],
                                 attenT[:, rb, :], start=(rb == 0), stop=(rb == RT - 1))
            otmp = small_p.tile([128, B_LOC], F32, tag="otmp", name=f"otmp_{ob}")
            nc.scalar.activation(otmp[:], pso[:], AF.Tanh,
                                 bias=b_o[:, ob:ob + 1], scale=1.0)
            ptb = ps_small.tile([128, 128], F32, tag="pssm", name=f"ptb_{ob}")
            nc.tensor.transpose(ptb[:], otmp[:], ident_f32[:])
            ostg = small_p.tile([128, 128], F32, tag="ostg", name=f"ostg_{ob}")
            nc.vector.tensor_copy(ostg[:], ptb[:])
            nc.gpsimd.dma_start(out=out_d[:, ob * 128:(ob + 1) * 128], in_=ostg[:])

    nc.compile()
    return nc


# ---------------- host side ----------------
_NC_CACHE = None


def _get_nc():
    global _NC_CACHE
    if _NC_CACHE is None:
        _NC_CACHE = build_nc()
    return _NC_CACHE


def prep_shared(W_ae, b_ae, W_c, b_c, W_s, b_s, W_h, b_h, W_al, b_al, W_o, b_o):
    bf = ml_dtypes.bfloat16
    f8 = ml_dtypes.float8_e4m3

    # DoubleRow pair-packed weights, f-index permuted for the u16 xbar:
    #   w_ae[p, fb, two, r] = 128 * W_ae.T[256*fb + 2*p + two, r]
    wt = np.ascontiguousarray(np.asarray(W_ae, np.float32).T * W_AE_SCALE)
    w_ae_t = np.ascontiguousarray(
        wt.reshape(FB, 128, 2, R).transpose(1, 0, 2, 3)).astype(f8)

    def pair_pack(w, scale):  # [p, t, two, n] = scale * w.T[(2t+two)*128 + p, n]
        wT = np.ascontiguousarray(np.asarray(w, np.float32).T * scale)
        return np.ascontiguousarray(
            wT.reshape(2, 2, 128, wT.shape[1]).transpose(2, 0, 1, 3)).astype(f8)

    def bt(b, nt):  # [p, t] = b[128*t + p]
        return np.ascontiguousarray(
            np.asarray(b, np.float32).reshape(nt, 128).T).astype(np.float32)

    wal_flat = np.asarray(W_al, np.float32)[0] * W_AL_SCALE  # [A]
    w_al_t = np.zeros((128, 2, 2, 16), dtype=f8)
    w_al_t[:, :, :, 0] = wal_flat.reshape(2, 2, 128).transpose(2, 0, 1).astype(f8)

    woT = np.ascontiguousarray(np.asarray(W_o, np.float32).T)
    w_o_t = np.ascontiguousarray(
        woT.reshape(RT, 128, R).transpose(1, 0, 2)).astype(bf)

    b_c = np.asarray(b_c, np.float32)
    b_s = np.asarray(b_s, np.float32)
    b_h = np.asarray(b_h, np.float32)

    return {
        "w_ae": w_ae_t,
        "w_c": pair_pack(W_c, W_C_SCALE),
        "w_s": pair_pack(W_s, W_C_SCALE),
        "w_h": pair_pack(W_h, W_C_SCALE),
        "w_al": w_al_t,
        "w_o": w_o_t,
        "b_ae": bt(b_ae, RT),
        "b_ch": bt(b_c + b_h, AT),
        "b_sh": bt(b_s + b_h, AT),
        "b_o": bt(b_o, RT),
        "ident": np.eye(128, dtype=bf),
        "ident_f32": np.eye(128, dtype=np.float32),
        "ones_row": np.ones((1, 128), dtype=bf),
    }


def make_in_maps(h, sentinel, att_feats, shared):
    h = np.asarray(h, np.float32)
    sentinel = np.asarray(sentinel, np.float32)
    att_feats = np.asarray(att_feats, np.float32)
    in_maps = []
    for i in range(NCORES):
        sl = slice(i * B_LOC, (i + 1) * B_LOC)
        m = dict(shared)
        m["h"] = np.ascontiguousarray(h[sl])
        m["sentinel"] = np.ascontiguousarray(sentinel[sl])
        m["att_feats"] = np.ascontiguousarray(att_feats[sl])
        in_maps.append(m)
    return in_maps


def kernel(h, sentinel, att_feats, W_ae, b_ae, W_c, b_c, W_s, b_s,
           W_h, b_h, W_al, b_al, W_o, b_o):
    shared = prep_shared(W_ae, b_ae, W_c, b_c, W_s, b_s, W_h, b_h, W_al, b_al, W_o, b_o)
    in_maps = make_in_maps(h, sentinel, att_feats, shared)
    nc = _get_nc()
    from concourse.bass_utils import run_bass_kernel_spmd
    res = run_bass_kernel_spmd(nc, in_maps, core_ids=list(range(NCORES)))
    out = np.concatenate([res.results[i]["out"] for i in range(NCORES)], axis=0)
    return np.ascontiguousarray(out.astype(np.float32))


if __name__ == "__main__":
    build_nc()
    print("built ok")
